# revision 16
# baseline (speedup 1.0000x reference)
"""Trainium2 Bass kernel for nn_PredLayer (soft gather / one-hot scatter of
per-class ConvLSTM states).

Full-input contract: kernel(**inputs) takes the unsharded numpy inputs and
returns (gathered_h, gathered_c, updated_h, updated_c) matching reference().

Sharding: data-parallel over the batch axis (bs=16 -> 2 per core, 8 cores).
Per-core device kernel streams (128, 3072) f32 tiles (h on partitions,
w*oc chunked by 2):
  gather:  acc = sum_c w[b,c] * states[c,b]      (tensor_scalar + 3 STT FMAs)
  scatter: upd = states*(1-m[c,b]) + new*m[c,b]  (ACT mul + STT FMA)
The scatter mask m is an exact one-hot in f32 (beta=1e10), so the blend is
bitwise-equal to the reference's A*(1-m)+B*m. All per-core divergence
(weights/mask values) is input data, so one SPMD program serves all cores.
"""

import numpy as np

NCLS, BS, H, W, OC = 4, 16, 128, 128, 48
N_CORES = 8
BS_LOCAL = BS // N_CORES          # 2
F = W * OC                        # 6144 f32 per h-row
N_WCHUNK = 2
FCH = F // N_WCHUNK               # 3072
GATHER_BETA = 3.0
SCATTER_BETA = 1e10

_COMPILED = None
LAST_RESULTS = None


def _softmax_f32(z):
    z = np.asarray(z, dtype=np.float32)
    e = np.exp(z - z.max(axis=-1, keepdims=True))
    return (e / e.sum(axis=-1, keepdims=True)).astype(np.float32)


def _build_bass():
    import concourse.bacc as bacc
    import concourse.mybir as mybir
    import concourse.tile as tile

    dt = mybir.dt.float32
    mult = mybir.AluOpType.mult
    add = mybir.AluOpType.add

    # Bacc (not Bass): its finalize() runs generate_event_semaphores, which
    # splits multi-sem waits — walrus rejects >1 sync wait per instruction.
    nc = bacc.Bacc(None, target_bir_lowering=False)
    sh = nc.dram_tensor("sh", [NCLS, BS_LOCAL, H, F], dt, kind="ExternalInput")
    sc = nc.dram_tensor("sc", [NCLS, BS_LOCAL, H, F], dt, kind="ExternalInput")
    nh = nc.dram_tensor("nh", [BS_LOCAL, H, F], dt, kind="ExternalInput")
    ncn = nc.dram_tensor("ncn", [BS_LOCAL, H, F], dt, kind="ExternalInput")
    wm = nc.dram_tensor("wm", [128, BS_LOCAL * NCLS * 3], dt, kind="ExternalInput")
    gh = nc.dram_tensor("gh", [BS_LOCAL, H, F], dt, kind="ExternalOutput")
    gc = nc.dram_tensor("gc", [BS_LOCAL, H, F], dt, kind="ExternalOutput")
    uh = nc.dram_tensor("uh", [NCLS, BS_LOCAL, H, F], dt, kind="ExternalOutput")
    uc = nc.dram_tensor("uc", [NCLS, BS_LOCAL, H, F], dt, kind="ExternalOutput")

    with tile.TileContext(nc) as tc:
        with (
            tc.tile_pool(name="sp", bufs=8) as sp,
            tc.tile_pool(name="npool", bufs=3) as npool,
            tc.tile_pool(name="tp", bufs=3) as tp,
            tc.tile_pool(name="ap", bufs=2) as ap,
            tc.tile_pool(name="wp", bufs=1) as wp,
        ):
            wmt = wp.tile([128, BS_LOCAL * NCLS * 3], dt, tag="wmt")
            nc.sync.dma_start(wmt[:], wm[:])
            gi = 0
            for b in range(BS_LOCAL):
                for s_in, n_in, g_out, u_out in ((sh, nh, gh, uh), (sc, ncn, gc, uc)):
                    for wi in range(N_WCHUNK):
                        # Alternate which HWDGE ring loads vs stores each
                        # group so both rings stay fed through ramp and drain.
                        ld, so = (nc.sync, nc.scalar) if gi % 2 == 0 else (nc.scalar, nc.sync)
                        gi += 1
                        w0 = wi * FCH
                        ntile = npool.tile([128, FCH], dt, tag="ntile")
                        ld.dma_start(ntile[:], n_in[b, :, w0 : w0 + FCH])
                        acc = ap.tile([128, FCH], dt, tag="acc")
                        for c in range(NCLS):
                            st = sp.tile([128, FCH], dt, tag="st")
                            ld.dma_start(st[:], s_in[c, b, :, w0 : w0 + FCH])
                            j = (b * NCLS + c) * 3
                            w_ap = wmt[:, j : j + 1]
                            m_ap = wmt[:, j + 1 : j + 2]
                            om_ap = wmt[:, j + 2 : j + 3]
                            if c == 0:
                                nc.vector.tensor_scalar_mul(acc[:], st[:], w_ap)
                            else:
                                nc.vector.scalar_tensor_tensor(
                                    acc[:], st[:], w_ap, acc[:], mult, add
                                )
                            tmp = tp.tile([128, FCH], dt, tag="tmp")
                            nc.scalar.mul(tmp[:], ntile[:], m_ap)
                            nc.vector.scalar_tensor_tensor(
                                st[:], st[:], om_ap, tmp[:], mult, add
                            )
                            so.dma_start(u_out[c, b, :, w0 : w0 + FCH], st[:])
                        so.dma_start(g_out[b, :, w0 : w0 + FCH], acc[:])
    # run_bass_via_pjrt doesn't finalize; Bacc needs it for alloc_regs +
    # generate_event_semaphores before serialization.
    nc.finalize()
    return nc


def _get_compiled():
    global _COMPILED
    if _COMPILED is None:
        _COMPILED = _build_bass()
    return _COMPILED


def kernel(states_h, states_c, new_h, new_c, logits, _trace=False, **_trace_kwargs):
    global LAST_RESULTS
    from concourse.bass_utils import run_bass_kernel_spmd

    states_h = np.asarray(states_h, dtype=np.float32).reshape(NCLS, BS, H, F)
    states_c = np.asarray(states_c, dtype=np.float32).reshape(NCLS, BS, H, F)
    new_h = np.asarray(new_h, dtype=np.float32).reshape(BS, H, F)
    new_c = np.asarray(new_c, dtype=np.float32).reshape(BS, H, F)
    logits = np.asarray(logits, dtype=np.float32)

    w = _softmax_f32(logits * np.float32(GATHER_BETA))        # (bs, ncls)
    m = _softmax_f32(logits * np.float32(SCATTER_BETA))       # (bs, ncls) ~one-hot

    in_maps = []
    for k in range(N_CORES):
        b0 = k * BS_LOCAL
        bsl = slice(b0, b0 + BS_LOCAL)
        vals = np.empty((BS_LOCAL * NCLS * 3,), dtype=np.float32)
        for bl in range(BS_LOCAL):
            for c in range(NCLS):
                j = (bl * NCLS + c) * 3
                vals[j] = w[b0 + bl, c]
                vals[j + 1] = m[b0 + bl, c]
                vals[j + 2] = np.float32(1.0) - m[b0 + bl, c]
        in_maps.append(
            {
                "sh": np.ascontiguousarray(states_h[:, bsl]),
                "sc": np.ascontiguousarray(states_c[:, bsl]),
                "nh": np.ascontiguousarray(new_h[bsl]),
                "ncn": np.ascontiguousarray(new_c[bsl]),
                "wm": np.ascontiguousarray(np.broadcast_to(vals, (128, vals.size))),
            }
        )

    nc = _get_compiled()
    res = run_bass_kernel_spmd(
        nc, in_maps, core_ids=list(range(N_CORES)), trace=_trace, **_trace_kwargs
    )
    LAST_RESULTS = res
    outs = res.results

    gathered_h = np.concatenate([outs[k]["gh"] for k in range(N_CORES)], axis=0)
    gathered_c = np.concatenate([outs[k]["gc"] for k in range(N_CORES)], axis=0)
    updated_h = np.concatenate([outs[k]["uh"] for k in range(N_CORES)], axis=1)
    updated_c = np.concatenate([outs[k]["uc"] for k in range(N_CORES)], axis=1)

    return (
        gathered_h.reshape(BS, H, W, OC),
        gathered_c.reshape(BS, H, W, OC),
        updated_h.reshape(NCLS, BS, H, W, OC),
        updated_c.reshape(NCLS, BS, H, W, OC),
    )


# revision 17
# speedup vs baseline: 1.1170x; 1.1170x over previous
"""Trainium2 Bass kernel for nn_PredLayer (soft gather / one-hot scatter of
per-class ConvLSTM states).

Full-input contract: kernel(**inputs) takes the unsharded numpy inputs and
returns (gathered_h, gathered_c, updated_h, updated_c) matching reference().

Sharding: data-parallel over the batch axis (bs=16 -> 2 per core, 8 cores).
Per-core device kernel streams (128, 3072) f32 tiles (h on partitions,
w*oc chunked by 2):
  gather:  acc = sum_c w[b,c] * states[c,b]      (tensor_scalar + 3 STT FMAs)
  scatter: upd = states*(1-m[c,b]) + new*m[c,b]  (ACT mul + STT FMA)
The scatter mask m is an exact one-hot in f32 (beta=1e10), so the blend is
bitwise-equal to the reference's A*(1-m)+B*m. All per-core divergence
(weights/mask values) is input data, so one SPMD program serves all cores.
"""

import numpy as np

NCLS, BS, H, W, OC = 4, 16, 128, 128, 48
N_CORES = 8
BS_LOCAL = BS // N_CORES          # 2
F = W * OC                        # 6144 f32 per h-row
N_WCHUNK = 2
FCH = F // N_WCHUNK               # 3072
GATHER_BETA = 3.0
SCATTER_BETA = 1e10

_COMPILED = None
LAST_RESULTS = None


def _softmax_f32(z):
    z = np.asarray(z, dtype=np.float32)
    e = np.exp(z - z.max(axis=-1, keepdims=True))
    return (e / e.sum(axis=-1, keepdims=True)).astype(np.float32)


def _build_bass():
    import concourse.bacc as bacc
    import concourse.mybir as mybir
    import concourse.tile as tile

    dt = mybir.dt.float32
    mult = mybir.AluOpType.mult
    add = mybir.AluOpType.add

    # Bacc (not Bass): its finalize() runs generate_event_semaphores, which
    # splits multi-sem waits — walrus rejects >1 sync wait per instruction.
    nc = bacc.Bacc(None, target_bir_lowering=False)
    sh = nc.dram_tensor("sh", [NCLS, BS_LOCAL, H, F], dt, kind="ExternalInput")
    sc = nc.dram_tensor("sc", [NCLS, BS_LOCAL, H, F], dt, kind="ExternalInput")
    nh = nc.dram_tensor("nh", [BS_LOCAL, H, F], dt, kind="ExternalInput")
    ncn = nc.dram_tensor("ncn", [BS_LOCAL, H, F], dt, kind="ExternalInput")
    wm = nc.dram_tensor("wm", [128, BS_LOCAL * NCLS * 3], dt, kind="ExternalInput")
    gh = nc.dram_tensor("gh", [BS_LOCAL, H, F], dt, kind="ExternalOutput")
    gc = nc.dram_tensor("gc", [BS_LOCAL, H, F], dt, kind="ExternalOutput")
    uh = nc.dram_tensor("uh", [NCLS, BS_LOCAL, H, F], dt, kind="ExternalOutput")
    uc = nc.dram_tensor("uc", [NCLS, BS_LOCAL, H, F], dt, kind="ExternalOutput")

    with tile.TileContext(nc) as tc:
        with (
            tc.tile_pool(name="sp", bufs=8) as sp,
            tc.tile_pool(name="npool", bufs=3) as npool,
            tc.tile_pool(name="tp", bufs=3) as tp,
            tc.tile_pool(name="ap", bufs=2) as ap,
            tc.tile_pool(name="wp", bufs=1) as wp,
        ):
            wmt = wp.tile([128, BS_LOCAL * NCLS * 3], dt, tag="wmt")
            nc.sync.dma_start(wmt[:], wm[:])
            for b in range(BS_LOCAL):
                for s_in, n_in, g_out, u_out in ((sh, nh, gh, uh), (sc, ncn, gc, uc)):
                    for wi in range(N_WCHUNK):
                        w0 = wi * FCH
                        ntile = npool.tile([128, FCH], dt, tag="ntile")
                        nc.sync.dma_start(ntile[:], n_in[b, :, w0 : w0 + FCH])
                        acc = ap.tile([128, FCH], dt, tag="acc")
                        for c in range(NCLS):
                            st = sp.tile([128, FCH], dt, tag="st")
                            nc.sync.dma_start(st[:], s_in[c, b, :, w0 : w0 + FCH])
                            j = (b * NCLS + c) * 3
                            w_ap = wmt[:, j : j + 1]
                            m_ap = wmt[:, j + 1 : j + 2]
                            om_ap = wmt[:, j + 2 : j + 3]
                            if c == 0:
                                nc.vector.tensor_scalar_mul(acc[:], st[:], w_ap)
                            else:
                                nc.vector.scalar_tensor_tensor(
                                    acc[:], st[:], w_ap, acc[:], mult, add
                                )
                            tmp = tp.tile([128, FCH], dt, tag="tmp")
                            nc.scalar.mul(tmp[:], ntile[:], m_ap)
                            nc.vector.scalar_tensor_tensor(
                                st[:], st[:], om_ap, tmp[:], mult, add
                            )
                            nc.scalar.dma_start(u_out[c, b, :, w0 : w0 + FCH], st[:])
                        nc.scalar.dma_start(g_out[b, :, w0 : w0 + FCH], acc[:])
    # run_bass_via_pjrt doesn't finalize; Bacc needs it for alloc_regs +
    # generate_event_semaphores before serialization.
    nc.finalize()
    return nc


def _get_compiled():
    global _COMPILED
    if _COMPILED is None:
        _COMPILED = _build_bass()
    return _COMPILED


def kernel(states_h, states_c, new_h, new_c, logits, _trace=False, **_trace_kwargs):
    global LAST_RESULTS
    from concourse.bass_utils import run_bass_kernel_spmd

    states_h = np.asarray(states_h, dtype=np.float32).reshape(NCLS, BS, H, F)
    states_c = np.asarray(states_c, dtype=np.float32).reshape(NCLS, BS, H, F)
    new_h = np.asarray(new_h, dtype=np.float32).reshape(BS, H, F)
    new_c = np.asarray(new_c, dtype=np.float32).reshape(BS, H, F)
    logits = np.asarray(logits, dtype=np.float32)

    w = _softmax_f32(logits * np.float32(GATHER_BETA))        # (bs, ncls)
    m = _softmax_f32(logits * np.float32(SCATTER_BETA))       # (bs, ncls) ~one-hot

    in_maps = []
    for k in range(N_CORES):
        b0 = k * BS_LOCAL
        bsl = slice(b0, b0 + BS_LOCAL)
        vals = np.empty((BS_LOCAL * NCLS * 3,), dtype=np.float32)
        for bl in range(BS_LOCAL):
            for c in range(NCLS):
                j = (bl * NCLS + c) * 3
                vals[j] = w[b0 + bl, c]
                vals[j + 1] = m[b0 + bl, c]
                vals[j + 2] = np.float32(1.0) - m[b0 + bl, c]
        in_maps.append(
            {
                "sh": np.ascontiguousarray(states_h[:, bsl]),
                "sc": np.ascontiguousarray(states_c[:, bsl]),
                "nh": np.ascontiguousarray(new_h[bsl]),
                "ncn": np.ascontiguousarray(new_c[bsl]),
                "wm": np.ascontiguousarray(np.broadcast_to(vals, (128, vals.size))),
            }
        )

    nc = _get_compiled()
    res = run_bass_kernel_spmd(
        nc, in_maps, core_ids=list(range(N_CORES)), trace=_trace, **_trace_kwargs
    )
    LAST_RESULTS = res
    outs = res.results

    gathered_h = np.concatenate([outs[k]["gh"] for k in range(N_CORES)], axis=0)
    gathered_c = np.concatenate([outs[k]["gc"] for k in range(N_CORES)], axis=0)
    updated_h = np.concatenate([outs[k]["uh"] for k in range(N_CORES)], axis=1)
    updated_c = np.concatenate([outs[k]["uc"] for k in range(N_CORES)], axis=1)

    return (
        gathered_h.reshape(BS, H, W, OC),
        gathered_c.reshape(BS, H, W, OC),
        updated_h.reshape(NCLS, BS, H, W, OC),
        updated_c.reshape(NCLS, BS, H, W, OC),
    )
